# revision 8
# baseline (speedup 1.0000x reference)
"""MixIT loss kernel for Trainium2 (8 NeuronCores, Bass/Tile) — v3.

Math: reference computes, for each of 16 assignment combinations k,
    mix[k,b,c,t] = sum_s A[k,c,s] * x[b,s,t]        (A tiny [16,2,4])
    loss[k] = sum_b [ snr(mix[k,b,0], m1[b]) + snr(mix[k,b,1], m2[b]) ]
and returns (argmin_k, min_k).  Everything reduces to the 6x6 Gram matrix
of the per-batch streams {x_0..x_3, m1, m2} over T=64000; the device
computes pairwise dot products, the host finishes the 16-way argmin.

v3 layout per core (4 batches = 24 streams; T = 128 partitions x 500 cols):
 - m1/m2 land as full-row f32 DMAs (2000B descriptors, ~94% HBM eff) on
   the scalar HWDGE ring; x lands in tapered T-chunks on the sync ring so
   the PE can chase; the tiny last chunk shrinks the serial tail.
 - Re-layout in two cheap stages instead of one slow strided cast:
     (1) contiguous cast copy za_f32 -> za_bf (f32->bf16, 2x DVE mode)
     (2) int64-packed transpose: zb[p, g, j, 0:4] = za_bf[p, j, 4g:4g+4].
         One i64 = 4 bf16 cols, so stage 2 moves 4x fewer elements and
         its destination runs are contiguous 256B lines.
 - PE: operand zb[:, g, :, :] = [128, 32x4] bf16 (24 real + 8 zeroed
   lanes) -> FWL fast weight load; 125 accumulating matmuls, 2 PSUM banks
   (bank A drained while the tail chunk computes).
 - Host sums the e-diagonal: G[j,k] = sum_banks sum_e out[(j,e),(k,e)].
"""

import itertools
import sys

import numpy as np

if "/opt/trn_rl_repo" not in sys.path:
    sys.path.insert(0, "/opt/trn_rl_repo")

N_CORES = 8
B = 32               # full batch
S = 4                # estimated sources
T = 64000
BL = B // N_CORES    # batches per core = 4
NJ = 6 * BL          # real streams per core = 24 (16 x, 4 m1, 4 m2)
NJP = 32             # padded lane count (FWL wants a 128-wide stationary)
P = 128
COLS = T // P        # 500
FG = 4               # T-cols fused per matmul: FG*NJP = 128
NG = COLS // FG      # 125 matmul groups
# x T-chunks (cols, each % FG == 0, sum == COLS).  Tapered: big chunks
# amortize DMA descriptors, the small last chunk shrinks the PE tail.
X_CHUNKS = (168, 140, 108, 64, 20)
assert sum(X_CHUNKS) == COLS and all(c % FG == 0 for c in X_CHUNKS)
SNR_MAX = 30.0

_CACHE = {}
LAST_RESULTS = None  # BassKernelResults of the most recent run (for test harness)


def _build_nc():
    from concourse import bacc, bass, tile
    import concourse.mybir as mybir

    nc = bacc.Bacc("TRN2", target_bir_lowering=False, debug=False,
                   num_devices=N_CORES)
    f32 = mybir.dt.float32
    bf16 = mybir.dt.bfloat16
    u32 = mybir.dt.uint32
    x = nc.dram_tensor("x", [BL, S, T], f32, kind="ExternalInput")
    m1 = nc.dram_tensor("m1", [BL, T], f32, kind="ExternalInput")
    m2 = nc.dram_tensor("m2", [BL, T], f32, kind="ExternalInput")
    g = nc.dram_tensor("g", [2, P, P], f32, kind="ExternalOutput")

    grp_a_end = sum(X_CHUNKS[:-1]) // FG       # bank A: all but last chunk

    with tile.TileContext(nc) as tc:
        with (
            tc.tile_pool(name="za", bufs=1) as zapool,
            tc.tile_pool(name="zc", bufs=1) as zcpool,
            tc.tile_pool(name="zb", bufs=1) as zbpool,
            tc.tile_pool(name="ps", bufs=1, space=bass.MemorySpace.PSUM) as psp,
            tc.tile_pool(name="o", bufs=1) as opool,
        ):
            za = zapool.tile([P, NJ, NG, FG], f32, tag="za")
            zc = zcpool.tile([P, NJ, NG, FG], bf16, tag="zc")
            zb = zbpool.tile([P, NG, NJP, FG], bf16, tag="zb")
            acc_a = psp.tile([P, P], f32, tag="pa")
            acc_b = psp.tile([P, P], f32, tag="pb")

            # Junk lanes 24:32 feed the matmul; zero them once (cheap, off
            # the critical path) so no NaNs/denormals hit the PE.
            nc.gpsimd.memset(zb[:, :, NJ:NJP, :].bitcast(u32), 0)

            xr = x.ap().rearrange("b s (p g e) -> p (b s) g e", p=P, e=FG)
            # m1/m2: full 500-col rows = 2000B descriptors, ~94% HBM eff.
            nc.scalar.dma_start(
                out=za[:, 16:20],
                in_=m1.ap().rearrange("b (p g e) -> p b g e", p=P, e=FG))
            nc.scalar.dma_start(
                out=za[:, 20:24],
                in_=m2.ap().rearrange("b (p g e) -> p b g e", p=P, e=FG))
            # x: T-chunked on the sync ring, 2 DMAs (8 streams) per chunk.
            g0 = 0
            for cq in X_CHUNKS:
                gq = cq // FG
                nc.sync.dma_start(out=za[:, 0:8, g0:g0 + gq],
                                  in_=xr[:, 0:8, g0:g0 + gq])
                nc.sync.dma_start(out=za[:, 8:16, g0:g0 + gq],
                                  in_=xr[:, 8:16, g0:g0 + gq])
                g0 += gq

            def cast(eng, j0, j1, g0, g1):
                eng.tensor_copy(zc[:, j0:j1, g0:g1], za[:, j0:j1, g0:g1])

            def transp(eng, j0, j1, g0, g1):
                # zb[p, g, j, :] = zc[p, j, g, :] as packed u32 pairs;
                # iterate (g, j, e2): contiguous 8B-per-j dst runs.
                dst = zb[:, g0:g1, j0:j1, :].bitcast(u32)
                src = zc[:, j0:j1, g0:g1].bitcast(u32).transpose([0, 2, 1, 3])
                eng.tensor_copy(dst, src)

            # m lanes (arrive earliest; DVE handles them up front)
            cast(nc.vector, 16, 24, 0, NG)
            transp(nc.vector, 16, 24, 0, NG)
            # x lanes per chunk: DVE takes 0:8, GpSimd takes 8:16.
            g0 = 0
            for cq in X_CHUNKS:
                gq = cq // FG
                cast(nc.vector, 0, 8, g0, g0 + gq)
                transp(nc.vector, 0, 8, g0, g0 + gq)
                cast(nc.gpsimd, 8, 16, g0, g0 + gq)
                transp(nc.gpsimd, 8, 16, g0, g0 + gq)
                g0 += gq

            for grp in range(NG):
                op = zb[:, grp, :, :]
                acc = acc_a if grp < grp_a_end else acc_b
                nc.tensor.matmul(
                    acc[:, :], op, op,
                    start=(grp == 0 or grp == grp_a_end),
                    stop=(grp == grp_a_end - 1 or grp == NG - 1),
                )
                if grp == grp_a_end - 1:
                    # bank A done: drain it while the PE runs the tail chunk
                    gout_a = opool.tile([P, P], f32, tag="oa")
                    nc.vector.tensor_copy(gout_a[:, :], acc_a[:, :])
                    nc.sync.dma_start(out=g.ap()[0], in_=gout_a[:, :])
            gout_b = opool.tile([P, P], f32, tag="ob")
            nc.vector.tensor_copy(gout_b[:, :], acc_b[:, :])
            nc.sync.dma_start(out=g.ap()[1], in_=gout_b[:, :])
    nc.compile()
    return nc


def _get_nc():
    if "nc" not in _CACHE:
        _CACHE["nc"] = _build_nc()
    return _CACHE["nc"]


def _finish_host(grams: np.ndarray):
    """grams: [N_CORES, 2, 128, 128] per-core PE blocks -> (argmin, min)."""
    # PSUM index = (lane j, c-within e), j-major.  Collapse banks and the
    # e-diagonal: G[j,k] = sum_banks sum_e out[(j,e),(k,e)], j,k in [0,24).
    g6 = grams.reshape(N_CORES, 2, NJP, FG, NJP, FG).astype(np.float64)
    g24 = np.einsum("cnjeke->cjk", g6)[:, :NJ, :NJ]

    # Per full-batch index b: core c = b // BL, local l = b % BL.
    # Stream layout per core: x_(l,s) at 4*l+s, m1_l at 16+l, m2_l at 20+l.
    Gxx = np.empty((B, S, S), np.float64)   # sum_t x_s x_s'
    C1 = np.empty((B, S), np.float64)       # sum_t x_s m1
    C2 = np.empty((B, S), np.float64)
    M1 = np.empty((B,), np.float64)         # sum_t m1^2
    M2 = np.empty((B,), np.float64)
    for b in range(B):
        c, l = divmod(b, BL)
        gm = g24[c]
        xs = slice(S * l, S * l + S)
        Gxx[b] = gm[xs, xs]
        C1[b] = gm[xs, 16 + l]
        C2[b] = gm[xs, 20 + l]
        M1[b] = gm[16 + l, 16 + l]
        M2[b] = gm[20 + l, 20 + l]

    combos = np.array(list(itertools.product([0, 1], repeat=S)), np.float64)
    losses = np.zeros(len(combos), np.float64)
    with np.errstate(divide="ignore"):
        for w, cc, mm in ((combos, C1, M1), (1.0 - combos, C2, M2)):
            bq = np.einsum("ks,bst,kt->kb", w, Gxx, w)        # sum_t y^2
            aq = bq - 2.0 * (w @ cc.T) + mm[None, :]          # sum_t (y-m)^2
            losses += np.sum(10.0 * np.log10(aq + SNR_MAX * bq)
                             - 10.0 * np.log10(bq), axis=1)
    k = int(np.argmin(losses))
    return np.int32(k), np.float32(losses[k])


def _ensure_trace_hook_safe():
    """If BASS_TRACE is set but this image lacks antenv.axon_hooks, install a
    null hook module so run_bass_kernel_spmd degrades to an untraced run
    instead of crashing on the import."""
    try:
        import antenv.axon_hooks  # noqa: F401
    except ImportError:
        import types

        stub = types.ModuleType("antenv.axon_hooks")
        stub.get_axon_ntff_profile_hook = lambda: None
        stub.set_axon_ntff_profile_hook = lambda h: None
        sys.modules["antenv.axon_hooks"] = stub


def kernel(estimated_sources: np.ndarray, m1: np.ndarray, m2: np.ndarray):
    global LAST_RESULTS
    _ensure_trace_hook_safe()
    from concourse.bass_utils import run_bass_kernel_spmd

    x = np.ascontiguousarray(estimated_sources, dtype=np.float32)
    m1 = np.ascontiguousarray(m1, dtype=np.float32)
    m2 = np.ascontiguousarray(m2, dtype=np.float32)

    in_maps = []
    for c in range(N_CORES):
        sl = slice(BL * c, BL * (c + 1))
        in_maps.append({
            "x": np.ascontiguousarray(x[sl]),
            "m1": np.ascontiguousarray(m1[sl]),
            "m2": np.ascontiguousarray(m2[sl]),
        })

    nc = _get_nc()
    LAST_RESULTS = run_bass_kernel_spmd(nc, in_maps, list(range(N_CORES)))
    grams = np.stack([LAST_RESULTS.results[c]["g"] for c in range(N_CORES)])
    return _finish_host(grams)


# revision 11
# speedup vs baseline: 1.0668x; 1.0668x over previous
"""MixIT loss kernel for Trainium2 (8 NeuronCores, Bass/Tile) — v4.

Math: reference computes, for each of 16 assignment combinations k,
    mix[k,b,c,t] = sum_s A[k,c,s] * x[b,s,t]        (A tiny [16,2,4])
    loss[k] = sum_b [ snr(mix[k,b,0], m1[b]) + snr(mix[k,b,1], m2[b]) ]
and returns (argmin_k, min_k).  Everything reduces to the 6x6 Gram matrix
of the per-batch streams {x_0..x_3, m1, m2} over T=64000; the device
computes pairwise dot products, the host finishes the 16-way argmin.

v5 layout per core (4 batches = 24 streams; T = 128 partitions x 100
groups x 5 cols):
 - za is a bf16 tile [128, 24, 100, 5, 2]; its f32 bitcast view is the
   DMA destination.  m1/m2 arrive as full-row DMAs (2000B descriptors)
   on the scalar ring; x arrives in tapered T-chunks on the sync ring.
 - The re-layout copies move ONLY the high u16 half of each f32 word
   (= the value truncated to bf16; error < 0.4%, cancels in the final
   log-ratio — validated ~3e-8 end-to-end) into the f-major tile
   zbt[128, g, e, j].  Plain bf16->bf16 copies (no cast opcode), spread
   over DVE (12 lanes) / GpSimd (8) / ACT (4, few big ops since each
   ACT op pays a ~1.4us pipe drain).
 - PE: 100 accumulating [128, 120]^2 bf16 matmuls (op = zbt[:, g]) into
   2 PSUM banks; bank A drains while the PE runs the small tail chunk.
 - Host sums the e-diagonal: G[j,k] = sum_banks sum_e out[(e,j),(e,k)].
"""

import itertools
import sys

import numpy as np

if "/opt/trn_rl_repo" not in sys.path:
    sys.path.insert(0, "/opt/trn_rl_repo")

N_CORES = 8
B = 32               # full batch
S = 4                # estimated sources
T = 64000
BL = B // N_CORES    # batches per core = 4
NJ = 6 * BL          # streams per core = 24 (16 x, 4 m1, 4 m2)
P = 128
COLS = T // P        # 500
FG = 5               # T-cols fused per matmul: FG*NJ = 120 <= 128
NG = COLS // FG      # 100 matmul groups
# x T-chunks (cols, each % FG == 0, sum == COLS).  Tapered: big chunks
# amortize DMA descriptors, the small last chunk shrinks the PE tail.
X_CHUNKS = (170, 140, 105, 65, 20)
assert sum(X_CHUNKS) == COLS and all(c % FG == 0 for c in X_CHUNKS)
SNR_MAX = 30.0

_CACHE = {}
LAST_RESULTS = None  # BassKernelResults of the most recent run (for test harness)


def _build_nc():
    from concourse import bacc, bass, tile
    import concourse.mybir as mybir

    nc = bacc.Bacc("TRN2", target_bir_lowering=False, debug=False,
                   num_devices=N_CORES)
    f32 = mybir.dt.float32
    bf16 = mybir.dt.bfloat16
    x = nc.dram_tensor("x", [BL, S, T], f32, kind="ExternalInput")
    m1 = nc.dram_tensor("m1", [BL, T], f32, kind="ExternalInput")
    m2 = nc.dram_tensor("m2", [BL, T], f32, kind="ExternalInput")
    g = nc.dram_tensor("g", [2, FG * NJ, FG * NJ], f32, kind="ExternalOutput")

    grp_a_end = sum(X_CHUNKS[:-1]) // FG       # bank A: all but last chunk

    with tile.TileContext(nc) as tc:
        with (
            tc.tile_pool(name="za", bufs=1) as zapool,
            tc.tile_pool(name="zb", bufs=1) as zbpool,
            tc.tile_pool(name="ps", bufs=1, space=bass.MemorySpace.PSUM) as psp,
            tc.tile_pool(name="o", bufs=1) as opool,
        ):
            za = zapool.tile([P, NJ, NG, FG, 2], bf16, tag="za")
            zf = za.bitcast(f32)               # [P, NJ, NG, FG, 1] f32 view
            zbt = zbpool.tile([P, NG, FG, NJ], bf16, tag="zbt")
            acc_a = psp.tile([FG * NJ, FG * NJ], f32, tag="pa")
            acc_b = psp.tile([FG * NJ, FG * NJ], f32, tag="pb")

            xr = x.ap().rearrange("b s (p g e) -> p (b s) g e", p=P, e=FG)
            # m1/m2: full 500-col rows = 2000B descriptors, ~94% HBM eff.
            nc.scalar.dma_start(
                out=zf[:, 16:20, :, :, 0],
                in_=m1.ap().rearrange("b (p g e) -> p b g e", p=P, e=FG))
            nc.scalar.dma_start(
                out=zf[:, 20:24, :, :, 0],
                in_=m2.ap().rearrange("b (p g e) -> p b g e", p=P, e=FG))
            # x: T-chunked on the sync ring, 2 DMAs (8 streams) per chunk.
            g0 = 0
            for cq in X_CHUNKS:
                gq = cq // FG
                nc.sync.dma_start(out=zf[:, 0:8, g0:g0 + gq, :, 0],
                                  in_=xr[:, 0:8, g0:g0 + gq])
                nc.sync.dma_start(out=zf[:, 8:16, g0:g0 + gq, :, 0],
                                  in_=xr[:, 8:16, g0:g0 + gq])
                g0 += gq

            def transp(eng, j0, j1, g0, g1):
                # zbt[p, g, e, j] = za[p, j, g, e, 1] (high u16 = bf16
                # truncation); iterate (g, e, j) to match dst layout.
                dst = zbt[:, g0:g1, :, j0:j1]
                src = za[:, j0:j1, g0:g1, :, 1].transpose([0, 2, 3, 1])
                if eng is nc.scalar:
                    eng.copy(dst, src)
                else:
                    eng.tensor_copy(dst, src)

            # ACT: m2 lanes in 3 chunk-aligned big ops (one drain each).
            transp(nc.scalar, 20, 24, 0, 34)
            transp(nc.scalar, 20, 24, 34, 68)
            transp(nc.scalar, 20, 24, 68, NG)
            g0 = 0
            for cq in X_CHUNKS:
                gq = cq // FG
                transp(nc.vector, 0, 8, g0, g0 + gq)    # x lanes 0:8
                transp(nc.gpsimd, 8, 16, g0, g0 + gq)   # x lanes 8:16
                transp(nc.vector, 16, 20, g0, g0 + gq)  # m1 lanes
                g0 += gq

            for grp in range(NG):
                op = zbt[:, grp]               # [P, FG, NJ] contiguous bf16
                acc = acc_a if grp < grp_a_end else acc_b
                nc.tensor.matmul(
                    acc[:, :], op, op,
                    start=(grp == 0 or grp == grp_a_end),
                    stop=(grp == grp_a_end - 1 or grp == NG - 1),
                )
                if grp == grp_a_end - 1:
                    # bank A done: drain it while the PE runs the tail chunk
                    gout_a = opool.tile([FG * NJ, FG * NJ], f32, tag="oa")
                    nc.vector.tensor_copy(gout_a[:, :], acc_a[:, :])
                    nc.sync.dma_start(out=g.ap()[0], in_=gout_a[:, :])
            gout_b = opool.tile([FG * NJ, FG * NJ], f32, tag="ob")
            nc.vector.tensor_copy(gout_b[:, :], acc_b[:, :])
            nc.sync.dma_start(out=g.ap()[1], in_=gout_b[:, :])
    nc.compile()
    return nc


def _get_nc():
    if "nc" not in _CACHE:
        _CACHE["nc"] = _build_nc()
    return _CACHE["nc"]


def _finish_host(grams: np.ndarray):
    """grams: [N_CORES, 2, 120, 120] per-core PE blocks -> (argmin, min)."""
    # PSUM index = (c-within e, lane j), e-major.  Collapse banks and the
    # e-diagonal: G[j,k] = sum_banks sum_e out[(e,j),(e,k)].
    g6 = grams.reshape(N_CORES, 2, FG, NJ, FG, NJ).astype(np.float64)
    g24 = np.einsum("cnejek->cjk", g6)

    # Per full-batch index b: core c = b // BL, local l = b % BL.
    # Stream layout per core: x_(l,s) at 4*l+s, m1_l at 16+l, m2_l at 20+l.
    Gxx = np.empty((B, S, S), np.float64)   # sum_t x_s x_s'
    C1 = np.empty((B, S), np.float64)       # sum_t x_s m1
    C2 = np.empty((B, S), np.float64)
    M1 = np.empty((B,), np.float64)         # sum_t m1^2
    M2 = np.empty((B,), np.float64)
    for b in range(B):
        c, l = divmod(b, BL)
        gm = g24[c]
        xs = slice(S * l, S * l + S)
        Gxx[b] = gm[xs, xs]
        C1[b] = gm[xs, 16 + l]
        C2[b] = gm[xs, 20 + l]
        M1[b] = gm[16 + l, 16 + l]
        M2[b] = gm[20 + l, 20 + l]

    combos = np.array(list(itertools.product([0, 1], repeat=S)), np.float64)
    losses = np.zeros(len(combos), np.float64)
    with np.errstate(divide="ignore"):
        for w, cc, mm in ((combos, C1, M1), (1.0 - combos, C2, M2)):
            bq = np.einsum("ks,bst,kt->kb", w, Gxx, w)        # sum_t y^2
            aq = bq - 2.0 * (w @ cc.T) + mm[None, :]          # sum_t (y-m)^2
            losses += np.sum(10.0 * np.log10(aq + SNR_MAX * bq)
                             - 10.0 * np.log10(bq), axis=1)
    k = int(np.argmin(losses))
    return np.int32(k), np.float32(losses[k])


def _ensure_trace_hook_safe():
    """If BASS_TRACE is set but this image lacks antenv.axon_hooks, install a
    null hook module so run_bass_kernel_spmd degrades to an untraced run
    instead of crashing on the import."""
    try:
        import antenv.axon_hooks  # noqa: F401
    except ImportError:
        import types

        stub = types.ModuleType("antenv.axon_hooks")
        stub.get_axon_ntff_profile_hook = lambda: None
        stub.set_axon_ntff_profile_hook = lambda h: None
        sys.modules["antenv.axon_hooks"] = stub


def kernel(estimated_sources: np.ndarray, m1: np.ndarray, m2: np.ndarray):
    global LAST_RESULTS
    _ensure_trace_hook_safe()
    from concourse.bass_utils import run_bass_kernel_spmd

    x = np.ascontiguousarray(estimated_sources, dtype=np.float32)
    m1 = np.ascontiguousarray(m1, dtype=np.float32)
    m2 = np.ascontiguousarray(m2, dtype=np.float32)

    in_maps = []
    for c in range(N_CORES):
        sl = slice(BL * c, BL * (c + 1))
        in_maps.append({
            "x": np.ascontiguousarray(x[sl]),
            "m1": np.ascontiguousarray(m1[sl]),
            "m2": np.ascontiguousarray(m2[sl]),
        })

    nc = _get_nc()
    LAST_RESULTS = run_bass_kernel_spmd(nc, in_maps, list(range(N_CORES)))
    grams = np.stack([LAST_RESULTS.results[c]["g"] for c in range(N_CORES)])
    return _finish_host(grams)


# revision 12
# speedup vs baseline: 1.3494x; 1.2649x over previous
"""MixIT loss kernel for Trainium2 (8 NeuronCores, Bass/Tile) — v6.

Math: reference computes, for each of 16 assignment combinations k,
    mix[k,b,c,t] = sum_s A[k,c,s] * x[b,s,t]        (A tiny [16,2,4])
    loss[k] = sum_b [ snr(mix[k,b,0], m1[b]) + snr(mix[k,b,1], m2[b]) ]
and returns (argmin_k, min_k).  Everything reduces to the 6x6 Gram matrix
of the per-batch streams {x_0..x_3, m1, m2} over T=64000; the device
computes pairwise dot products, the host finishes the 16-way argmin.

v6 layout per core (4 batches = 24 streams; T = 128 partitions x 500
cols):
 - DMA (f32): m1/m2 as full-row transfers (2000B descriptors, ~94% HBM
   eff) on the scalar ring; x in tapered T-chunks on the sync ring so
   downstream stages can chase; the tiny last chunk shrinks the tail.
 - Re-layout: f32 tensor_copy za[p, j, c] -> zbt[p, c, j] (the fastest
   measured DVE pattern: strided 4B reads, contiguous 96B dst runs, 24
   lanes per op).  Work is split along c across DVE / GpSimd / ACT in
   per-chunk slices sized to each engine's measured rate; ACT gets one
   big op per chunk since each ACT op pays a ~1.4us pipe drain.
 - PE reads zbt as bf16 with NO cast: the operand is the high u16 half
   of each f32 word (= bf16 truncation, error < 0.4%, cancels in the
   final log-ratio; validated ~3e-8 end-to-end) via a uniform stride-2
   view — [128, 120] per group, so matmuls run 1-pass bf16 instead of
   2-pass fp32.  100 accumulating matmuls into 2 PSUM banks; bank A
   drains while the PE runs the small tail chunk.
 - Host sums the e-diagonal: G[j,k] = sum_banks sum_e out[(e,j),(e,k)].
"""

import itertools
import sys

import numpy as np

if "/opt/trn_rl_repo" not in sys.path:
    sys.path.insert(0, "/opt/trn_rl_repo")

N_CORES = 8
B = 32               # full batch
S = 4                # estimated sources
T = 64000
BL = B // N_CORES    # batches per core = 4
NJ = 6 * BL          # streams per core = 24 (16 x, 4 m1, 4 m2)
P = 128
COLS = T // P        # 500
FG = 5               # T-cols fused per matmul: FG*NJ = 120 <= 128
NG = COLS // FG      # 100 matmul groups
# x T-chunks (cols, each % FG == 0, sum == COLS).  Tapered: big chunks
# amortize DMA descriptors, the small last chunk shrinks the PE tail.
X_CHUNKS = (170, 140, 105, 65, 20)
assert sum(X_CHUNKS) == COLS and all(c % FG == 0 for c in X_CHUNKS)
# Per-chunk copy split across engines, fractions of the chunk's columns;
# sized to measured rates DVE 2.0 / GpSimd 3.5 / ACT 3.6 ns/elem.
CP_DVE, CP_GPS = 0.47, 0.28           # ACT gets the rest
SNR_MAX = 30.0

_CACHE = {}
LAST_RESULTS = None  # BassKernelResults of the most recent run (for test harness)


def _build_nc():
    from concourse import bacc, bass, tile
    import concourse.mybir as mybir

    nc = bacc.Bacc("TRN2", target_bir_lowering=False, debug=False,
                   num_devices=N_CORES)
    f32 = mybir.dt.float32
    bf16 = mybir.dt.bfloat16
    x = nc.dram_tensor("x", [BL, S, T], f32, kind="ExternalInput")
    m1 = nc.dram_tensor("m1", [BL, T], f32, kind="ExternalInput")
    m2 = nc.dram_tensor("m2", [BL, T], f32, kind="ExternalInput")
    g = nc.dram_tensor("g", [2, FG * NJ, FG * NJ], f32, kind="ExternalOutput")

    grp_a_end = sum(X_CHUNKS[:-1]) // FG       # bank A: all but last chunk

    with tile.TileContext(nc) as tc:
        with (
            tc.tile_pool(name="za", bufs=1) as zapool,
            tc.tile_pool(name="zb", bufs=1) as zbpool,
            tc.tile_pool(name="ps", bufs=1, space=bass.MemorySpace.PSUM) as psp,
            tc.tile_pool(name="o", bufs=1) as opool,
        ):
            za = zapool.tile([P, NJ, COLS], f32, tag="za")
            zbt = zbpool.tile([P, COLS, NJ, 2], bf16, tag="zbt")
            zbf = zbt.bitcast(f32)             # [P, COLS, NJ, 1] f32 view
            acc_a = psp.tile([FG * NJ, FG * NJ], f32, tag="pa")
            acc_b = psp.tile([FG * NJ, FG * NJ], f32, tag="pb")

            xr = x.ap().rearrange("b s (p c) -> p (b s) c", p=P)
            # m1/m2: full 500-col rows = 2000B descriptors, ~94% HBM eff.
            nc.scalar.dma_start(
                out=za[:, 16:20, :],
                in_=m1.ap().rearrange("b (p c) -> p b c", p=P))
            nc.scalar.dma_start(
                out=za[:, 20:24, :],
                in_=m2.ap().rearrange("b (p c) -> p b c", p=P))
            # x: T-chunked on the sync ring, 2 DMAs (8 streams) per chunk.
            c0 = 0
            for cq in X_CHUNKS:
                nc.sync.dma_start(out=za[:, 0:8, c0:c0 + cq],
                                  in_=xr[:, 0:8, c0:c0 + cq])
                nc.sync.dma_start(out=za[:, 8:16, c0:c0 + cq],
                                  in_=xr[:, 8:16, c0:c0 + cq])
                c0 += cq

            def cp(eng, c0, c1):
                if c1 <= c0:
                    return
                dst = zbf[:, c0:c1, :, 0]
                src = za[:, :, c0:c1].transpose([0, 2, 1])
                if eng is nc.scalar:
                    eng.copy(dst, src)
                else:
                    eng.tensor_copy(dst, src)

            # Copies chase each chunk; c-slices per engine, 24-lane ops.
            c0 = 0
            for cq in X_CHUNKS:
                dv = c0 + int(cq * CP_DVE)
                gp = dv + int(cq * CP_GPS)
                cp(nc.vector, c0, dv)
                cp(nc.gpsimd, dv, gp)
                cp(nc.scalar, gp, c0 + cq)
                c0 += cq

            for grp in range(NG):
                # High u16 halves of 120 consecutive f32: [128, 120] bf16
                # at uniform element stride 2 — a 1-pass bf16 matmul.
                op = zbt[:, FG * grp:FG * (grp + 1), :, 1]
                acc = acc_a if grp < grp_a_end else acc_b
                nc.tensor.matmul(
                    acc[:, :], op, op,
                    start=(grp == 0 or grp == grp_a_end),
                    stop=(grp == grp_a_end - 1 or grp == NG - 1),
                )
                if grp == grp_a_end - 1:
                    # bank A done: drain it while the PE runs the tail chunk
                    gout_a = opool.tile([FG * NJ, FG * NJ], f32, tag="oa")
                    nc.vector.tensor_copy(gout_a[:, :], acc_a[:, :])
                    nc.sync.dma_start(out=g.ap()[0], in_=gout_a[:, :])
            gout_b = opool.tile([FG * NJ, FG * NJ], f32, tag="ob")
            nc.vector.tensor_copy(gout_b[:, :], acc_b[:, :])
            nc.sync.dma_start(out=g.ap()[1], in_=gout_b[:, :])
    nc.compile()
    return nc


def _get_nc():
    if "nc" not in _CACHE:
        _CACHE["nc"] = _build_nc()
    return _CACHE["nc"]


def _finish_host(grams: np.ndarray):
    """grams: [N_CORES, 2, 120, 120] per-core PE blocks -> (argmin, min)."""
    # PSUM index = (c-within e, lane j), e-major.  Collapse banks and the
    # e-diagonal: G[j,k] = sum_banks sum_e out[(e,j),(e,k)].
    g6 = grams.reshape(N_CORES, 2, FG, NJ, FG, NJ).astype(np.float64)
    g24 = np.einsum("cnejek->cjk", g6)

    # Per full-batch index b: core c = b // BL, local l = b % BL.
    # Stream layout per core: x_(l,s) at 4*l+s, m1_l at 16+l, m2_l at 20+l.
    Gxx = np.empty((B, S, S), np.float64)   # sum_t x_s x_s'
    C1 = np.empty((B, S), np.float64)       # sum_t x_s m1
    C2 = np.empty((B, S), np.float64)
    M1 = np.empty((B,), np.float64)         # sum_t m1^2
    M2 = np.empty((B,), np.float64)
    for b in range(B):
        c, l = divmod(b, BL)
        gm = g24[c]
        xs = slice(S * l, S * l + S)
        Gxx[b] = gm[xs, xs]
        C1[b] = gm[xs, 16 + l]
        C2[b] = gm[xs, 20 + l]
        M1[b] = gm[16 + l, 16 + l]
        M2[b] = gm[20 + l, 20 + l]

    combos = np.array(list(itertools.product([0, 1], repeat=S)), np.float64)
    losses = np.zeros(len(combos), np.float64)
    with np.errstate(divide="ignore"):
        for w, cc, mm in ((combos, C1, M1), (1.0 - combos, C2, M2)):
            bq = np.einsum("ks,bst,kt->kb", w, Gxx, w)        # sum_t y^2
            aq = bq - 2.0 * (w @ cc.T) + mm[None, :]          # sum_t (y-m)^2
            losses += np.sum(10.0 * np.log10(aq + SNR_MAX * bq)
                             - 10.0 * np.log10(bq), axis=1)
    k = int(np.argmin(losses))
    return np.int32(k), np.float32(losses[k])


def _ensure_trace_hook_safe():
    """If BASS_TRACE is set but this image lacks antenv.axon_hooks, install a
    null hook module so run_bass_kernel_spmd degrades to an untraced run
    instead of crashing on the import."""
    try:
        import antenv.axon_hooks  # noqa: F401
    except ImportError:
        import types

        stub = types.ModuleType("antenv.axon_hooks")
        stub.get_axon_ntff_profile_hook = lambda: None
        stub.set_axon_ntff_profile_hook = lambda h: None
        sys.modules["antenv.axon_hooks"] = stub


def kernel(estimated_sources: np.ndarray, m1: np.ndarray, m2: np.ndarray):
    global LAST_RESULTS
    _ensure_trace_hook_safe()
    from concourse.bass_utils import run_bass_kernel_spmd

    x = np.ascontiguousarray(estimated_sources, dtype=np.float32)
    m1 = np.ascontiguousarray(m1, dtype=np.float32)
    m2 = np.ascontiguousarray(m2, dtype=np.float32)

    in_maps = []
    for c in range(N_CORES):
        sl = slice(BL * c, BL * (c + 1))
        in_maps.append({
            "x": np.ascontiguousarray(x[sl]),
            "m1": np.ascontiguousarray(m1[sl]),
            "m2": np.ascontiguousarray(m2[sl]),
        })

    nc = _get_nc()
    LAST_RESULTS = run_bass_kernel_spmd(nc, in_maps, list(range(N_CORES)))
    grams = np.stack([LAST_RESULTS.results[c]["g"] for c in range(N_CORES)])
    return _finish_host(grams)
